# revision 13
# baseline (speedup 1.0000x reference)
"""DeltaNet chunkwise delta-rule kernel for Trainium2 (8 NeuronCores).

Math (per (b,h) pair, raw-input formulation; all per-token scalings folded
into per-partition ops):
  qn = Dg q, kn = Dc k  (Dg=1/|q|, Dc=1/|k|, norms with eps inside sqrt)
  A  = Da K K^T Dc (Da = beta*Dc), T = (I + strict(A))^-1
  u0 = T Dbeta v ; w = T Da k ; attn = tril(qn kn^T)
  scan: u = u0 - w S ; o = qn S + attn u ; S += kn^T u

Change of variables u' = Dc u makes everything computable from RAW q,k,v
with only per-token ROW scalings:
  P = K K^T (raw),  einv_i = beta_i/(|k_i|^2+eps),  a_i = beta_i/sqrt(|k_i|^2+eps)
  B = -einv ∘ strict(P)       (row-scaled strict lower)
  TnD = (I+B)(I+B^2)(I+B^4)(I+B^8)   (exact to ~4e-5 for c=128 blocks)
  [U0'|W'] = TnD [a∘v | einv∘k]
  u' = U0' - W' S ; o = g ∘ (Q S + tril(Q K^T) u') ; S += K^T u'
Chunk size C=128 (math is chunk-size invariant vs reference's c=32).
"""

import numpy as np
from contextlib import ExitStack

PAIRS = 8      # (b,h) pairs per core
L = 4096
D = 64         # dk == dv
C = 128        # macro-chunk = partition tile
NT = L // C    # 32 tiles per pair
EPS = 1e-6

_CACHED = {}
LAST_RESULT = None


def _hoist_waits(nc, mybir):
    """This container's walrus rejects semaphore waits embedded on compute
    instructions; move them onto preceding EventSemaphore instructions
    (<=2 waits each) on the same engine."""
    n = 0
    for f in nc.m.functions:
        for bb in f.blocks:
            newinsts = []
            for inst in bb.instructions:
                si = getattr(inst, "sync_info", None)
                ow = list(getattr(si, "on_wait", None) or []) if si else []
                if ow and not isinstance(inst, mybir.InstEventSemaphore):
                    for j in range(0, len(ow), 2):
                        ev = mybir.InstEventSemaphore(
                            name=f"{inst.name}-wh{j}", ins=[], outs=[],
                            engine=inst.engine)
                        ev.sync_info = mybir.SyncInfo(
                            on_wait=ow[j:j + 2], on_update=[])
                        newinsts.append(ev)
                        n += 1
                    si.on_wait = []
                newinsts.append(inst)
            bb.instructions = newinsts
    return n


def _build_nc(pairs=PAIRS, ntiles=NT):
    import concourse.bass as bass
    import concourse.tile as tile
    from concourse import mybir
    from concourse.masks import make_identity

    FP32 = mybir.dt.float32
    BF16 = mybir.dt.bfloat16
    ALU = mybir.AluOpType
    ACTF = mybir.ActivationFunctionType
    Ltot = ntiles * C

    nc = bass.Bass("TRN2", use_seq_codegen=True)
    q_d = nc.dram_tensor("q", [pairs, Ltot, D], FP32, kind="ExternalInput")
    k_d = nc.dram_tensor("k", [pairs, Ltot, D], FP32, kind="ExternalInput")
    v_d = nc.dram_tensor("v", [pairs, Ltot, D], FP32, kind="ExternalInput")
    b_d = nc.dram_tensor("beta", [pairs, Ltot], FP32, kind="ExternalInput")
    o_d = nc.dram_tensor("o", [pairs, Ltot, D], FP32, kind="ExternalOutput")

    with tile.TileContext(nc) as tc, ExitStack() as ctx:
        singles = ctx.enter_context(tc.tile_pool(name="singles", bufs=1))
        slabs = ctx.enter_context(tc.tile_pool(name="slabs", bufs=2))
        small = ctx.enter_context(tc.tile_pool(name="small", bufs=2))
        work = ctx.enter_context(tc.tile_pool(name="work", bufs=3))
        scratch = ctx.enter_context(tc.tile_pool(name="scratch", bufs=2))
        p_tr = ctx.enter_context(tc.tile_pool(name="p_tr", bufs=1, space="PSUM"))
        p_pf = ctx.enter_context(tc.tile_pool(name="p_pf", bufs=2, space="PSUM"))
        p_sq = ctx.enter_context(tc.tile_pool(name="p_sq", bufs=2, space="PSUM"))
        p_sc = ctx.enter_context(tc.tile_pool(name="p_sc", bufs=2, space="PSUM"))
        p_S = ctx.enter_context(tc.tile_pool(name="p_S", bufs=1, space="PSUM"))

        # Constants
        ident_bf = singles.tile([C, C], BF16)
        make_identity(nc, ident_bf)
        ident_f = singles.tile([C, C], FP32)
        make_identity(nc, ident_f)
        eps_t = singles.tile([C, 1], FP32)
        nc.vector.memset(eps_t, EPS)

        for p in range(pairs):
            # ---- load slabs: [128 tok, NT, D] (token-within-tile on partitions)
            q_f = slabs.tile([C, ntiles, D], FP32, tag="q_f")
            k_f = slabs.tile([C, ntiles, D], FP32, tag="k_f")
            v_f = slabs.tile([C, ntiles, D], FP32, tag="v_f")
            beta = slabs.tile([C, ntiles], FP32, tag="beta")
            nc.sync.dma_start(out=q_f, in_=q_d[p].rearrange("(t c) d -> c t d", c=C))
            nc.sync.dma_start(out=k_f, in_=k_d[p].rearrange("(t c) d -> c t d", c=C))
            nc.sync.dma_start(out=v_f, in_=v_d[p].rearrange("(t c) d -> c t d", c=C))
            nc.gpsimd.dma_start(out=beta,
                                in_=b_d[p].rearrange("(t c) -> c t", c=C))

            # ---- per-pair prep (batched elementwise)
            sq = scratch.tile([C, ntiles, D], FP32, tag="sq")
            sumq = small.tile([C, ntiles], FP32, tag="sumq")
            nc.scalar.square(sq, q_f)
            nc.vector.tensor_reduce(out=sumq, in_=sq, axis=mybir.AxisListType.X,
                                    op=ALU.add)
            g = small.tile([C, ntiles], FP32, tag="g")  # 1/sqrt(sum q^2+eps)
            nc.scalar.activation(out=g, in_=sumq, func=ACTF.Sqrt, bias=eps_t)
            nc.vector.reciprocal(out=g, in_=g)

            sk = scratch.tile([C, ntiles, D], FP32, tag="sq")
            sumk = small.tile([C, ntiles], FP32, tag="sumk")
            nc.scalar.square(sk, k_f)
            nc.vector.tensor_reduce(out=sumk, in_=sk, axis=mybir.AxisListType.X,
                                    op=ALU.add)
            rk = small.tile([C, ntiles], FP32, tag="rk")  # 1/sqrt(sum k^2+eps)
            nc.scalar.activation(out=rk, in_=sumk, func=ACTF.Sqrt, bias=eps_t)
            nc.vector.reciprocal(out=rk, in_=rk)
            a_s = small.tile([C, ntiles], FP32, tag="a_s")      # beta/|k|
            nc.vector.tensor_tensor(out=a_s, in0=beta, in1=rk, op=ALU.mult)
            einv = small.tile([C, ntiles], FP32, tag="einv")    # beta/|k|^2
            nc.vector.tensor_tensor(out=einv, in0=a_s, in1=rk, op=ALU.mult)
            einvn = small.tile([C, ntiles], FP32, tag="einvn")  # -beta/|k|^2
            nc.scalar.mul(out=einvn, in_=einv, mul=-1.0)

            # X' = [a∘v | einv∘k] bf16; q,k bf16 casts
            X_bf = slabs.tile([C, ntiles, 2 * D], BF16, tag="X_bf")
            a_b = bass.AP(tensor=a_s.tensor, offset=a_s.offset,
                          ap=[*a_s.ap, [0, D]])
            e_b = bass.AP(tensor=einv.tensor, offset=einv.offset,
                          ap=[*einv.ap, [0, D]])
            nc.vector.tensor_tensor(out=X_bf[:, :, 0:D], in0=v_f, in1=a_b,
                                    op=ALU.mult)
            nc.vector.tensor_tensor(out=X_bf[:, :, D:2 * D], in0=k_f, in1=e_b,
                                    op=ALU.mult)
            q_bf = slabs.tile([C, ntiles, D], BF16, tag="q_bf")
            k_bf = slabs.tile([C, ntiles, D], BF16, tag="k_bf")
            nc.scalar.copy(out=q_bf, in_=q_f)
            nc.scalar.copy(out=k_bf, in_=k_f)

            o_f = slabs.tile([C, ntiles, D], FP32, tag="o_f")

            S_psum = p_S.tile([D, D], FP32)
            S_bf = None

            for t in range(ntiles):
                # transposes
                pt_q = p_tr.tile([D, C], BF16, tag="pt")
                nc.tensor.transpose(pt_q, q_bf[:, t, :], ident_bf)
                qT = work.tile([D, C], BF16, tag="qT")
                nc.scalar.copy(out=qT, in_=pt_q)
                pt_k = p_tr.tile([D, C], BF16, tag="pt")
                nc.tensor.transpose(pt_k, k_bf[:, t, :], ident_bf)
                kT = work.tile([D, C], BF16, tag="kT")
                nc.scalar.copy(out=kT, in_=pt_k)

                # P = K K^T ; Fraw = K Q^T  (full 128x128)
                ps_P = p_pf.tile([C, C], FP32, tag="ps_pf")
                nc.tensor.matmul(ps_P, lhsT=kT, rhs=kT, start=True, stop=True)
                ps_F = p_pf.tile([C, C], FP32, tag="ps_pf")
                nc.tensor.matmul(ps_F, lhsT=kT, rhs=qT, start=True, stop=True)

                # B = -einv ∘ strict_lower(P)   (bf16)
                tmpB = work.tile([C, C], FP32, tag="tmpB")
                nc.vector.tensor_scalar(out=tmpB, in0=ps_P,
                                        scalar1=einvn[:, t:t + 1], scalar2=None,
                                        op0=ALU.mult)
                B_bf = work.tile([C, C], BF16, tag="B_bf")
                nc.gpsimd.affine_select(out=B_bf, in_=tmpB,
                                        compare_op=ALU.is_gt, fill=0.0, base=0,
                                        pattern=[[-1, C]], channel_multiplier=1)

                # Fm = triu_incl_diag(K Q^T) = (tril(Q K^T))^T   (bf16)
                F_pre = work.tile([C, C], BF16, tag="F_pre")
                nc.scalar.copy(out=F_pre, in_=ps_F)
                Fm = work.tile([C, C], BF16, tag="Fm")
                nc.gpsimd.affine_select(out=Fm, in_=F_pre,
                                        compare_op=ALU.is_ge, fill=0.0, base=0,
                                        pattern=[[1, C]], channel_multiplier=-1)

                # B^T (raw) and M1T = I + B^T
                pt_B = p_tr.tile([C, C], BF16, tag="pt")
                nc.tensor.transpose(pt_B, B_bf, ident_bf)
                BT = work.tile([C, C], BF16, tag="BT")
                nc.scalar.copy(out=BT, in_=pt_B)
                M1T = work.tile([C, C], BF16, tag="M1T")
                nc.vector.tensor_tensor(out=M1T, in0=pt_B, in1=ident_f,
                                        op=ALU.add)

                # squarings: B2|B2T, B4|B4T, B8
                ps_ab = p_sq.tile([C, 2 * C], FP32, tag="ps_sq")
                nc.tensor.matmul(ps_ab[:, 0:C], lhsT=BT, rhs=B_bf,
                                 start=True, stop=True)
                nc.tensor.matmul(ps_ab[:, C:2 * C], lhsT=B_bf, rhs=BT,
                                 start=True, stop=True)
                sq2 = work.tile([C, 2 * C], BF16, tag="sq2")
                nc.vector.tensor_copy(sq2, ps_ab)
                ps_cd = p_sq.tile([C, 2 * C], FP32, tag="ps_sq")
                nc.tensor.matmul(ps_cd[:, 0:C], lhsT=sq2[:, C:2 * C],
                                 rhs=sq2[:, 0:C], start=True, stop=True)
                nc.tensor.matmul(ps_cd[:, C:2 * C], lhsT=sq2[:, 0:C],
                                 rhs=sq2[:, C:2 * C], start=True, stop=True)
                sq4 = work.tile([C, 2 * C], BF16, tag="sq4")
                nc.vector.tensor_copy(sq4, ps_cd)
                ps_e = p_sq.tile([C, C], FP32, tag="ps_sq")
                nc.tensor.matmul(ps_e, lhsT=sq4[:, C:2 * C], rhs=sq4[:, 0:C],
                                 start=True, stop=True)
                sq8 = work.tile([C, C], BF16, tag="sq8")
                nc.vector.tensor_copy(sq8, ps_e)

                # R-chain: R1 = M2^T M1^T ; R2 = M4^T R1 ; R3 = M8^T R2 = TnD^T
                ps_r = p_sq.tile([C, C], FP32, tag="ps_sq")
                nc.tensor.matmul(ps_r, lhsT=sq2[:, 0:C], rhs=M1T,
                                 start=True, stop=False)
                nc.tensor.matmul(ps_r, lhsT=ident_bf, rhs=M1T,
                                 start=False, stop=True)
                R1 = work.tile([C, C], BF16, tag="R1")
                nc.scalar.copy(out=R1, in_=ps_r)
                ps_r2 = p_sq.tile([C, C], FP32, tag="ps_sq")
                nc.tensor.matmul(ps_r2, lhsT=sq4[:, 0:C], rhs=R1,
                                 start=True, stop=False)
                nc.tensor.matmul(ps_r2, lhsT=ident_bf, rhs=R1,
                                 start=False, stop=True)
                R2 = work.tile([C, C], BF16, tag="R2")
                nc.scalar.copy(out=R2, in_=ps_r2)
                ps_r3 = p_sq.tile([C, C], FP32, tag="ps_sq")
                nc.tensor.matmul(ps_r3, lhsT=sq8, rhs=R2,
                                 start=True, stop=False)
                nc.tensor.matmul(ps_r3, lhsT=ident_bf, rhs=R2,
                                 start=False, stop=True)
                R3 = work.tile([C, C], BF16, tag="R3")
                nc.scalar.copy(out=R3, in_=ps_r3)

                # Y = TnD X' = [U0' | W']
                ps_y = p_sq.tile([C, 2 * D], FP32, tag="ps_sq")
                nc.tensor.matmul(ps_y, lhsT=R3, rhs=X_bf[:, t, :],
                                 start=True, stop=True)
                Y = work.tile([C, 2 * D], BF16, tag="Y")
                nc.scalar.copy(out=Y, in_=ps_y)

                # W'^T
                pt_w = p_tr.tile([D, C], BF16, tag="pt")
                nc.tensor.transpose(pt_w, Y[:, D:2 * D], ident_bf)
                WT = work.tile([D, C], BF16, tag="WT")
                nc.scalar.copy(out=WT, in_=pt_w)

                # scan
                if t == 0:
                    u_bf = Y[:, 0:D]
                else:
                    ps_u = p_sc.tile([C, D], FP32, tag="ps_sc")
                    nc.tensor.matmul(ps_u, lhsT=WT, rhs=S_bf,
                                     start=True, stop=True)
                    u_t = work.tile([C, D], BF16, tag="u_t")
                    nc.vector.tensor_tensor(out=u_t, in0=Y[:, 0:D], in1=ps_u,
                                            op=ALU.subtract)
                    u_bf = u_t

                ps_o = p_sc.tile([C, D], FP32, tag="ps_sc")
                if t == 0:
                    nc.tensor.matmul(ps_o, lhsT=Fm, rhs=u_bf,
                                     start=True, stop=True)
                else:
                    nc.tensor.matmul(ps_o, lhsT=qT, rhs=S_bf,
                                     start=True, stop=False)
                    nc.tensor.matmul(ps_o, lhsT=Fm, rhs=u_bf,
                                     start=False, stop=True)
                nc.scalar.activation(out=o_f[:, t, :], in_=ps_o,
                                     func=ACTF.Copy, scale=g[:, t:t + 1])

                # S += K^T u'
                nc.tensor.matmul(S_psum, lhsT=k_bf[:, t, :], rhs=u_bf,
                                 start=(t == 0), stop=(t == ntiles - 1),
                                 skip_group_check=True)
                if t < ntiles - 1:
                    S_new = work.tile([D, D], BF16, tag="S_bf")
                    nc.scalar.copy(out=S_new, in_=S_psum)
                    S_bf = S_new

            nc.sync.dma_start(out=o_d[p].rearrange("(t c) d -> c t d", c=C),
                              in_=o_f)

    _hoist_waits(nc, mybir)
    return nc


def _get_nc(pairs=PAIRS, ntiles=NT):
    key = (pairs, ntiles)
    if key not in _CACHED:
        _CACHED[key] = _build_nc(pairs, ntiles)
    return _CACHED[key]


def kernel(q, k, v, beta):
    from concourse.bass_utils import run_bass_kernel_spmd

    b, h, Lx, d = q.shape
    n_cores = 8
    bh = b * h
    per = bh // n_cores
    qf = np.ascontiguousarray(q.reshape(bh, Lx, d).astype(np.float32))
    kf = np.ascontiguousarray(k.reshape(bh, Lx, d).astype(np.float32))
    vf = np.ascontiguousarray(v.reshape(bh, Lx, d).astype(np.float32))
    bf = np.ascontiguousarray(beta.reshape(bh, Lx).astype(np.float32))

    nc = _get_nc(per, Lx // C)
    in_maps = []
    for c in range(n_cores):
        sl = slice(c * per, (c + 1) * per)
        in_maps.append({"q": qf[sl], "k": kf[sl], "v": vf[sl], "beta": bf[sl]})
    res = run_bass_kernel_spmd(nc, in_maps, core_ids=list(range(n_cores)))
    global LAST_RESULT
    LAST_RESULT = res
    o = np.concatenate([r["o"] for r in res.results], axis=0)
    return o.reshape(b, h, Lx, d).astype(np.float32)


# revision 16
# speedup vs baseline: 1.0674x; 1.0674x over previous
"""DeltaNet chunkwise delta-rule kernel for Trainium2 (8 NeuronCores).

Math (per (b,h) pair, raw-input formulation; all per-token scalings folded
into per-partition ops):
  qn = Dg q, kn = Dc k  (Dg=1/|q|, Dc=1/|k|, norms with eps inside sqrt)
  A  = Da K K^T Dc (Da = beta*Dc), T = (I + strict(A))^-1
  u0 = T Dbeta v ; w = T Da k ; attn = tril(qn kn^T)
  scan: u = u0 - w S ; o = qn S + attn u ; S += kn^T u

Change of variables u' = Dc u makes everything computable from RAW q,k,v
with only per-token ROW scalings:
  P = K K^T (raw),  einv_i = beta_i/(|k_i|^2+eps),  a_i = beta_i/sqrt(|k_i|^2+eps)
  B = -einv ∘ strict(P)       (row-scaled strict lower)
  TnD = (I+B)(I+B^2)(I+B^4)(I+B^8)   (exact to ~4e-5 for c=128 blocks)
  [U0'|W'] = TnD [a∘v | einv∘k]
  u' = U0' - W' S ; o = g ∘ (Q S + tril(Q K^T) u') ; S += K^T u'
Chunk size C=128 (math is chunk-size invariant vs reference's c=32).
"""

import numpy as np
from contextlib import ExitStack

PAIRS = 8      # (b,h) pairs per core
L = 4096
D = 64         # dk == dv
C = 128        # macro-chunk = partition tile
NT = L // C    # 32 tiles per pair
EPS = 1e-6

_CACHED = {}
LAST_RESULT = None


def _hoist_waits(nc, mybir):
    """This container's walrus rejects semaphore waits embedded on compute
    instructions; move them onto preceding EventSemaphore instructions
    (<=2 waits each) on the same engine."""
    n = 0
    for f in nc.m.functions:
        for bb in f.blocks:
            newinsts = []
            for inst in bb.instructions:
                si = getattr(inst, "sync_info", None)
                ow = list(getattr(si, "on_wait", None) or []) if si else []
                if ow and not isinstance(inst, mybir.InstEventSemaphore):
                    for j in range(0, len(ow), 2):
                        ev = mybir.InstEventSemaphore(
                            name=f"{inst.name}-wh{j}", ins=[], outs=[],
                            engine=inst.engine)
                        ev.sync_info = mybir.SyncInfo(
                            on_wait=ow[j:j + 2], on_update=[])
                        newinsts.append(ev)
                        n += 1
                    si.on_wait = []
                newinsts.append(inst)
            bb.instructions = newinsts
    return n


def _build_nc(pairs=PAIRS, ntiles=NT):
    import concourse.bass as bass
    import concourse.tile as tile
    from concourse import mybir
    from concourse.masks import make_identity

    FP32 = mybir.dt.float32
    BF16 = mybir.dt.bfloat16
    ALU = mybir.AluOpType
    ACTF = mybir.ActivationFunctionType
    Ltot = ntiles * C

    nc = bass.Bass("TRN2", use_seq_codegen=True)
    q_d = nc.dram_tensor("q", [pairs, Ltot, D], FP32, kind="ExternalInput")
    k_d = nc.dram_tensor("k", [pairs, Ltot, D], FP32, kind="ExternalInput")
    v_d = nc.dram_tensor("v", [pairs, Ltot, D], FP32, kind="ExternalInput")
    b_d = nc.dram_tensor("beta", [pairs, Ltot], FP32, kind="ExternalInput")
    o_d = nc.dram_tensor("o", [pairs, Ltot, D], FP32, kind="ExternalOutput")

    with tile.TileContext(nc) as tc, ExitStack() as ctx:
        singles = ctx.enter_context(tc.tile_pool(name="singles", bufs=1))
        slabs = ctx.enter_context(tc.tile_pool(name="slabs", bufs=2))
        small = ctx.enter_context(tc.tile_pool(name="small", bufs=2))
        work = ctx.enter_context(tc.tile_pool(name="work", bufs=4))
        scratch = ctx.enter_context(tc.tile_pool(name="scratch", bufs=2))
        p_tr = ctx.enter_context(tc.tile_pool(name="p_tr", bufs=1, space="PSUM"))
        p_pf = ctx.enter_context(tc.tile_pool(name="p_pf", bufs=2, space="PSUM"))
        p_sq = ctx.enter_context(tc.tile_pool(name="p_sq", bufs=3, space="PSUM"))
        p_sc = ctx.enter_context(tc.tile_pool(name="p_sc", bufs=1, space="PSUM"))
        p_S = ctx.enter_context(tc.tile_pool(name="p_S", bufs=1, space="PSUM"))

        # Constants
        ident_bf = singles.tile([C, C], BF16)
        make_identity(nc, ident_bf)
        ident_f = singles.tile([C, C], FP32)
        make_identity(nc, ident_f)
        eps_t = singles.tile([C, 1], FP32)
        nc.vector.memset(eps_t, EPS)

        for p in range(pairs):
            # ---- load slabs: [128 tok, NT, D] (token-within-tile on partitions)
            q_f = slabs.tile([C, ntiles, D], FP32, tag="q_f")
            k_f = slabs.tile([C, ntiles, D], FP32, tag="k_f")
            v_f = slabs.tile([C, ntiles, D], FP32, tag="v_f")
            beta = slabs.tile([C, ntiles], FP32, tag="beta")
            nc.sync.dma_start(out=q_f, in_=q_d[p].rearrange("(t c) d -> c t d", c=C))
            nc.sync.dma_start(out=k_f, in_=k_d[p].rearrange("(t c) d -> c t d", c=C))
            nc.sync.dma_start(out=v_f, in_=v_d[p].rearrange("(t c) d -> c t d", c=C))
            nc.gpsimd.dma_start(out=beta,
                                in_=b_d[p].rearrange("(t c) -> c t", c=C))

            # ---- per-pair prep (batched elementwise)
            sq = scratch.tile([C, ntiles, D], FP32, tag="sq")
            sumq = small.tile([C, ntiles], FP32, tag="sumq")
            nc.scalar.square(sq, q_f)
            nc.vector.tensor_reduce(out=sumq, in_=sq, axis=mybir.AxisListType.X,
                                    op=ALU.add)
            g = small.tile([C, ntiles], FP32, tag="g")  # 1/sqrt(sum q^2+eps)
            nc.scalar.activation(out=g, in_=sumq, func=ACTF.Sqrt, bias=eps_t)
            nc.vector.reciprocal(out=g, in_=g)

            sk = scratch.tile([C, ntiles, D], FP32, tag="sq")
            sumk = small.tile([C, ntiles], FP32, tag="sumk")
            nc.scalar.square(sk, k_f)
            nc.vector.tensor_reduce(out=sumk, in_=sk, axis=mybir.AxisListType.X,
                                    op=ALU.add)
            rk = small.tile([C, ntiles], FP32, tag="rk")  # 1/sqrt(sum k^2+eps)
            nc.scalar.activation(out=rk, in_=sumk, func=ACTF.Sqrt, bias=eps_t)
            nc.vector.reciprocal(out=rk, in_=rk)
            a_s = small.tile([C, ntiles], FP32, tag="a_s")      # beta/|k|
            nc.vector.tensor_tensor(out=a_s, in0=beta, in1=rk, op=ALU.mult)
            einv = small.tile([C, ntiles], FP32, tag="einv")    # beta/|k|^2
            nc.vector.tensor_tensor(out=einv, in0=a_s, in1=rk, op=ALU.mult)
            einvn = small.tile([C, ntiles], FP32, tag="einvn")  # -beta/|k|^2
            nc.scalar.mul(out=einvn, in_=einv, mul=-1.0)

            # X' = [a∘v | einv∘k] bf16; q,k bf16 casts
            X_bf = slabs.tile([C, ntiles, 2 * D], BF16, tag="X_bf")
            a_b = bass.AP(tensor=a_s.tensor, offset=a_s.offset,
                          ap=[*a_s.ap, [0, D]])
            e_b = bass.AP(tensor=einv.tensor, offset=einv.offset,
                          ap=[*einv.ap, [0, D]])
            nc.vector.tensor_tensor(out=X_bf[:, :, 0:D], in0=v_f, in1=a_b,
                                    op=ALU.mult)
            nc.vector.tensor_tensor(out=X_bf[:, :, D:2 * D], in0=k_f, in1=e_b,
                                    op=ALU.mult)
            q_bf = slabs.tile([C, ntiles, D], BF16, tag="q_bf")
            k_bf = slabs.tile([C, ntiles, D], BF16, tag="k_bf")
            nc.scalar.copy(out=q_bf, in_=q_f)
            nc.scalar.copy(out=k_bf, in_=k_f)

            o_f = slabs.tile([C, ntiles, D], FP32, tag="o_f")

            S_psum = p_S.tile([D, D], FP32)
            S_bf = None

            for t in range(ntiles):
                # transposes
                pt_qk = p_tr.tile([D, 2 * C], BF16, tag="pt")
                nc.tensor.transpose(pt_qk[:, 0:C], q_bf[:, t, :], ident_bf)
                nc.tensor.transpose(pt_qk[:, C:2 * C], k_bf[:, t, :], ident_bf)
                qkT = work.tile([D, 2 * C], BF16, tag="qkT")
                nc.scalar.copy(out=qkT, in_=pt_qk)
                qT = qkT[:, 0:C]
                kT = qkT[:, C:2 * C]

                # P = K K^T ; Fraw = K Q^T  (full 128x128)
                ps_P = p_pf.tile([C, C], FP32, tag="ps_pf")
                nc.tensor.matmul(ps_P, lhsT=kT, rhs=kT, start=True, stop=True)
                ps_F = p_pf.tile([C, C], FP32, tag="ps_pf")
                nc.tensor.matmul(ps_F, lhsT=kT, rhs=qT, start=True, stop=True)

                # B = -einv ∘ strict_lower(P)   (bf16)
                tmpB = work.tile([C, C], FP32, tag="tmpB")
                nc.vector.tensor_scalar(out=tmpB, in0=ps_P,
                                        scalar1=einvn[:, t:t + 1], scalar2=None,
                                        op0=ALU.mult)
                B_bf = work.tile([C, C], BF16, tag="B_bf")
                nc.gpsimd.affine_select(out=B_bf, in_=tmpB,
                                        compare_op=ALU.is_gt, fill=0.0, base=0,
                                        pattern=[[-1, C]], channel_multiplier=1)

                # Fm = triu_incl_diag(K Q^T) = (tril(Q K^T))^T   (bf16)
                F_pre = work.tile([C, C], BF16, tag="F_pre")
                nc.scalar.copy(out=F_pre, in_=ps_F)
                Fm = work.tile([C, C], BF16, tag="Fm")
                nc.gpsimd.affine_select(out=Fm, in_=F_pre,
                                        compare_op=ALU.is_ge, fill=0.0, base=0,
                                        pattern=[[1, C]], channel_multiplier=-1)

                # B^T (raw) and M1T = I + B^T
                pt_B = p_tr.tile([C, C], BF16, tag="pt")
                nc.tensor.transpose(pt_B, B_bf, ident_bf)
                BT = work.tile([C, C], BF16, tag="BT")
                nc.scalar.copy(out=BT, in_=pt_B)
                M1T = work.tile([C, C], BF16, tag="M1T")
                nc.vector.tensor_tensor(out=M1T, in0=pt_B, in1=ident_f,
                                        op=ALU.add)

                # squarings: B2|B2T, B4|B4T, B8
                ps_ab = p_sq.tile([C, 2 * C], FP32, tag="ps_sq")
                nc.tensor.matmul(ps_ab[:, 0:C], lhsT=BT, rhs=B_bf,
                                 start=True, stop=True)
                nc.tensor.matmul(ps_ab[:, C:2 * C], lhsT=B_bf, rhs=BT,
                                 start=True, stop=True)
                sq2 = work.tile([C, 2 * C], BF16, tag="sq2")
                nc.vector.tensor_copy(sq2, ps_ab)
                ps_cd = p_sq.tile([C, 2 * C], FP32, tag="ps_sq")
                nc.tensor.matmul(ps_cd[:, 0:C], lhsT=sq2[:, C:2 * C],
                                 rhs=sq2[:, 0:C], start=True, stop=True)
                nc.tensor.matmul(ps_cd[:, C:2 * C], lhsT=sq2[:, 0:C],
                                 rhs=sq2[:, C:2 * C], start=True, stop=True)
                sq4 = work.tile([C, 2 * C], BF16, tag="sq4")
                nc.vector.tensor_copy(sq4, ps_cd)
                ps_e = p_sq.tile([C, C], FP32, tag="ps_sq")
                nc.tensor.matmul(ps_e, lhsT=sq4[:, C:2 * C], rhs=sq4[:, 0:C],
                                 start=True, stop=True)
                sq8 = work.tile([C, C], BF16, tag="sq8")
                nc.vector.tensor_copy(sq8, ps_e)

                # R-chain: R1 = M2^T M1^T ; R2 = M4^T R1 ; R3 = M8^T R2 = TnD^T
                ps_r = p_sq.tile([C, C], FP32, tag="ps_sq")
                nc.tensor.matmul(ps_r, lhsT=sq2[:, 0:C], rhs=M1T,
                                 start=True, stop=True)
                R1 = work.tile([C, C], BF16, tag="R1")
                nc.vector.tensor_tensor(out=R1, in0=ps_r, in1=M1T, op=ALU.add)
                ps_r2 = p_sq.tile([C, C], FP32, tag="ps_sq")
                nc.tensor.matmul(ps_r2, lhsT=sq4[:, 0:C], rhs=R1,
                                 start=True, stop=True)
                R2 = work.tile([C, C], BF16, tag="R2")
                nc.vector.tensor_tensor(out=R2, in0=ps_r2, in1=R1, op=ALU.add)
                ps_r3 = p_sq.tile([C, C], FP32, tag="ps_sq")
                nc.tensor.matmul(ps_r3, lhsT=sq8, rhs=R2,
                                 start=True, stop=True)
                R3 = work.tile([C, C], BF16, tag="R3")
                nc.vector.tensor_tensor(out=R3, in0=ps_r3, in1=R2, op=ALU.add)

                # Y = TnD X' = [U0' | W']
                ps_y = p_sq.tile([C, 2 * D], FP32, tag="ps_sq")
                nc.tensor.matmul(ps_y, lhsT=R3, rhs=X_bf[:, t, :],
                                 start=True, stop=True)
                Y = work.tile([C, 2 * D], BF16, tag="Y")
                nc.scalar.copy(out=Y, in_=ps_y)

                # W'^T
                pt_w = p_tr.tile([D, C], BF16, tag="pt")
                nc.tensor.transpose(pt_w, Y[:, D:2 * D], ident_bf)
                WT = work.tile([D, C], BF16, tag="WT")
                nc.scalar.copy(out=WT, in_=pt_w)

                # scan
                if t == 0:
                    u_bf = Y[:, 0:D]
                else:
                    ps_u = p_sc.tile([C, D], FP32, tag="ps_sc")
                    nc.tensor.matmul(ps_u, lhsT=WT, rhs=S_bf,
                                     start=True, stop=True)
                    u_t = work.tile([C, D], BF16, tag="u_t")
                    nc.vector.tensor_tensor(out=u_t, in0=Y[:, 0:D], in1=ps_u,
                                            op=ALU.subtract)
                    u_bf = u_t

                ps_o = p_sc.tile([C, D], FP32, tag="ps_sc")
                if t == 0:
                    nc.tensor.matmul(ps_o, lhsT=Fm, rhs=u_bf,
                                     start=True, stop=True)
                else:
                    nc.tensor.matmul(ps_o, lhsT=qT, rhs=S_bf,
                                     start=True, stop=False)
                    nc.tensor.matmul(ps_o, lhsT=Fm, rhs=u_bf,
                                     start=False, stop=True)
                nc.scalar.activation(out=o_f[:, t, :], in_=ps_o,
                                     func=ACTF.Copy, scale=g[:, t:t + 1])

                # S += K^T u'
                nc.tensor.matmul(S_psum, lhsT=k_bf[:, t, :], rhs=u_bf,
                                 start=(t == 0), stop=(t == ntiles - 1),
                                 skip_group_check=True)
                if t < ntiles - 1:
                    S_new = work.tile([D, D], BF16, tag="S_bf")
                    nc.scalar.copy(out=S_new, in_=S_psum)
                    S_bf = S_new

            nc.sync.dma_start(out=o_d[p].rearrange("(t c) d -> c t d", c=C),
                              in_=o_f)

    _hoist_waits(nc, mybir)
    return nc


def _get_nc(pairs=PAIRS, ntiles=NT):
    key = (pairs, ntiles)
    if key not in _CACHED:
        _CACHED[key] = _build_nc(pairs, ntiles)
    return _CACHED[key]


def kernel(q, k, v, beta):
    from concourse.bass_utils import run_bass_kernel_spmd

    b, h, Lx, d = q.shape
    n_cores = 8
    bh = b * h
    per = bh // n_cores
    qf = np.ascontiguousarray(q.reshape(bh, Lx, d).astype(np.float32))
    kf = np.ascontiguousarray(k.reshape(bh, Lx, d).astype(np.float32))
    vf = np.ascontiguousarray(v.reshape(bh, Lx, d).astype(np.float32))
    bf = np.ascontiguousarray(beta.reshape(bh, Lx).astype(np.float32))

    nc = _get_nc(per, Lx // C)
    in_maps = []
    for c in range(n_cores):
        sl = slice(c * per, (c + 1) * per)
        in_maps.append({"q": qf[sl], "k": kf[sl], "v": vf[sl], "beta": bf[sl]})
    res = run_bass_kernel_spmd(nc, in_maps, core_ids=list(range(n_cores)))
    global LAST_RESULT
    LAST_RESULT = res
    o = np.concatenate([r["o"] for r in res.results], axis=0)
    return o.reshape(b, h, Lx, d).astype(np.float32)
